# revision 6
# baseline (speedup 1.0000x reference)
"""Trainium2 Bass kernel for nn_CausalSelfAttention_70832600646065.

Sliding-window causal GQA attention (B=2, T=2048, C=1024, NH=16, NKV=4,
HD=64, window=1024) with RoPE + RMSNorm on q/k, a value-embedding gate, and
an output projection.

Sharding: sequence-parallel over 8 cores. Core c handles batch c//4, query
rows [512*(c%4), 512*(c%4)+512). Each core receives a transposed bf16 slice
of x covering its query rows plus a 1024-row key/value halo (zero-padded at
the sequence start), so no collectives are needed.

Per-core pipeline (all matmuls bf16 with fp32 PSUM accumulation):
  A1: K/V/gate projections from xT (stationary) per 128-row tile, RoPE +
      RMSNorm on K, PE-transpose K -> KT [d, seq]; V gated with ve and
      stored as [128, 4, 65] bf16 tiles whose 65th column is the per-key
      validity bit (0 for rows before the sequence start) used to build
      softmax denominators while neutralizing padded keys.
  A2: Q projection + RoPE + RMSNorm, PE-transpose -> QT [d, seq].
  B:  per (head, 128-row tile): 9 QK^T matmuls into a [128, 1152] PSUM
      scores strip (keys on partitions), one Exp activation (scale=1/8)
      into bf16 probabilities, static sliding-window/causal edge masks,
      9 accumulating AV matmuls -> [128, 65] (out | denominator), then a
      reciprocal + per-partition scale into Y.
  C:  PE-transpose Y -> YT, output projection, DMA out.

The softmax skips the max-subtraction: q/k are RMS-normalized so
|q.k|/8 <= 8 and exp() cannot overflow fp32.
"""

import sys

if "/opt/trn_rl_repo" not in sys.path:
    sys.path.insert(0, "/opt/trn_rl_repo")

import numpy as np
import ml_dtypes

import concourse.bass as bass
import concourse.bacc as bacc
import concourse.mybir as mybir
import concourse.tile as tile
from concourse.bass_utils import run_bass_kernel_spmd
from concourse.masks import make_identity

F32 = mybir.dt.float32
BF16 = mybir.dt.bfloat16
AF = mybir.ActivationFunctionType
OP = mybir.AluOpType

B, T, C = 2, 2048, 1024
NH, NKV, HD = 16, 4, 64
VEC = 32
WIN = 1024
QR = 512           # query rows per core
KR = QR + WIN      # key rows per core (incl. halo)
NQT = QR // 128    # 4 query row tiles
NKT = KR // 128    # 12 key row tiles
NCT = C // 128     # 8 contraction tiles
NJB = WIN // 128 + 1  # 9 key tiles in any 128-row query tile's window
EPS = float(np.finfo(np.float32).eps)
N_CORES = 8


def _rope_rms(nc, pools, src_psum, cos_t, sin_t, dst_sb, nh):
    """src_psum: [128, nh*64] fp32 PSUM view. dst_sb: [128, nh*64] bf16 SBUF.
    Applies rope (cos/sin [128, 32] tiles broadcast over heads) then rmsnorm."""
    rr = pools["rr"].tile([128, nh * HD], F32)
    tmp = pools["rtmp"].tile([128, nh * HD], F32)
    src = src_psum.rearrange("p (h d) -> p h d", h=nh)
    x1, x2 = src[:, :, 0:32], src[:, :, 32:64]
    r3 = rr[:].rearrange("p (h d) -> p h d", h=nh)
    t3 = tmp[:].rearrange("p (h d) -> p h d", h=nh)
    cosb = cos_t[:].unsqueeze(1).broadcast_to([128, nh, 32])
    sinb = sin_t[:].unsqueeze(1).broadcast_to([128, nh, 32])
    # rope: out1 = x1*cos + x2*sin ; out2 = x2*cos - x1*sin
    nc.vector.tensor_tensor(t3[:, :, 0:32], x1, cosb, op=OP.mult)
    nc.vector.tensor_tensor(t3[:, :, 32:64], x2, sinb, op=OP.mult)
    nc.vector.tensor_tensor(r3[:, :, 0:32], t3[:, :, 0:32], t3[:, :, 32:64], op=OP.add)
    nc.vector.tensor_tensor(t3[:, :, 0:32], x2, cosb, op=OP.mult)
    nc.vector.tensor_tensor(t3[:, :, 32:64], x1, sinb, op=OP.mult)
    nc.vector.tensor_tensor(
        r3[:, :, 32:64], t3[:, :, 0:32], t3[:, :, 32:64], op=OP.subtract
    )
    # rmsnorm: rinv = rsqrt(mean(rr^2) + eps), via exp(-0.5*ln(m)) + 1 Newton step
    nc.vector.tensor_tensor(tmp[:], rr[:], rr[:], op=OP.mult)
    ms = pools["ms"].tile([128, nh], F32)
    nc.vector.tensor_reduce(ms[:], t3[:, :, :], axis=mybir.AxisListType.X, op=OP.add)
    nc.vector.tensor_scalar(
        ms[:], ms[:], 1.0 / HD, EPS, op0=OP.mult, op1=OP.add
    )
    lnm = pools["ms"].tile([128, nh], F32, tag="lnm")
    nc.scalar.activation(lnm[:], ms[:], AF.Ln)
    r0 = pools["ms"].tile([128, nh], F32, tag="r0")
    nc.scalar.activation(r0[:], lnm[:], AF.Exp, scale=-0.5)
    # Newton: r1 = r0 * (1.5 - 0.5*m*r0^2)
    t0 = pools["ms"].tile([128, nh], F32, tag="t0")
    nc.vector.tensor_tensor(t0[:], r0[:], r0[:], op=OP.mult)
    nc.vector.tensor_tensor(t0[:], t0[:], ms[:], op=OP.mult)
    nc.vector.tensor_scalar(t0[:], t0[:], -0.5, 1.5, op0=OP.mult, op1=OP.add)
    nc.vector.tensor_tensor(r0[:], r0[:], t0[:], op=OP.mult)
    rb = r0[:].unsqueeze(2).broadcast_to([128, nh, HD])
    dst3 = dst_sb.rearrange("p (h d) -> p h d", h=nh)
    nc.vector.tensor_tensor(dst3, r3[:, :, :], rb, op=OP.mult)


def build_program():
    nc = bacc.Bacc("TRN2", target_bir_lowering=False, debug=False,
                   num_devices=N_CORES)

    xT = nc.declare_dram_parameter("xT", [C, KR], BF16, isOutput=False)
    ve_d = nc.declare_dram_parameter("ve", [KR, NKV * HD], BF16, isOutput=False)
    cos_d = nc.declare_dram_parameter("cos", [KR, 32], F32, isOutput=False)
    sin_d = nc.declare_dram_parameter("sin", [KR, 32], F32, isOutput=False)
    wq_d = nc.declare_dram_parameter("wq", [C, NH * HD], BF16, isOutput=False)
    wk_d = nc.declare_dram_parameter("wk", [C, NKV * HD], BF16, isOutput=False)
    wv_d = nc.declare_dram_parameter("wv", [C, NKV * HD], BF16, isOutput=False)
    wp_d = nc.declare_dram_parameter("wproj", [C, C], BF16, isOutput=False)
    wg_d = nc.declare_dram_parameter("wgate", [VEC, NKV], BF16, isOutput=False)
    valid_d = nc.declare_dram_parameter("valid", [NKT, 128, NKV], BF16,
                                        isOutput=False)
    y_d = nc.declare_dram_parameter("y", [QR, C], F32, isOutput=True)

    with tile.TileContext(nc) as tc:
        with (
            tc.tile_pool(name="wgt", bufs=1) as wgt,       # weights + inputs
            tc.tile_pool(name="persist", bufs=1) as persist,  # QT/KT/V/Y/YT
            tc.tile_pool(name="small", bufs=1) as small,
        ):
            # ---- input DMAs -------------------------------------------------
            xT_sb = []
            for ct in range(NCT):
                t = wgt.tile([128, KR], BF16, tag=f"xT{ct}")
                nc.sync.dma_start(t[:], xT.ap()[ct * 128:(ct + 1) * 128, :])
                xT_sb.append(t)
            wq_sb, wk_sb, wv_sb, wp_sb = [], [], [], []
            for ct in range(NCT):
                t = wgt.tile([128, NH * HD], BF16, tag=f"wq{ct}")
                nc.sync.dma_start(t[:], wq_d.ap()[ct * 128:(ct + 1) * 128, :])
                wq_sb.append(t)
                t = wgt.tile([128, NKV * HD], BF16, tag=f"wk{ct}")
                nc.sync.dma_start(t[:], wk_d.ap()[ct * 128:(ct + 1) * 128, :])
                wk_sb.append(t)
                t = wgt.tile([128, NKV * HD], BF16, tag=f"wv{ct}")
                nc.sync.dma_start(t[:], wv_d.ap()[ct * 128:(ct + 1) * 128, :])
                wv_sb.append(t)
                t = wgt.tile([128, C], BF16, tag=f"wp{ct}")
                nc.sync.dma_start(t[:], wp_d.ap()[ct * 128:(ct + 1) * 128, :])
                wp_sb.append(t)
            wg_sb = wgt.tile([VEC, NKV], BF16, tag="wg")
            nc.sync.dma_start(wg_sb[:], wg_d.ap())
            ve_sb, cos_sb, sin_sb = [], [], []
            for rt in range(NKT):
                t = wgt.tile([128, NKV * HD], BF16, tag=f"ve{rt}")
                nc.sync.dma_start(t[:], ve_d.ap()[rt * 128:(rt + 1) * 128, :])
                ve_sb.append(t)
                t = wgt.tile([128, 32], F32, tag=f"cos{rt}")
                nc.sync.dma_start(t[:], cos_d.ap()[rt * 128:(rt + 1) * 128, :])
                cos_sb.append(t)
                t = wgt.tile([128, 32], F32, tag=f"sin{rt}")
                nc.sync.dma_start(t[:], sin_d.ap()[rt * 128:(rt + 1) * 128, :])
                sin_sb.append(t)

            # identity (for PE transpose) and edge mask tiles
            ident = small.tile([128, 128], BF16, tag="ident")
            make_identity(nc, ident[:])
            # mask_lo: keep p >= f (window edge, jb==0)
            mask_lo = small.tile([128, 128], BF16, tag="mask_lo")
            nc.gpsimd.memset(mask_lo[:], 1.0)
            nc.gpsimd.affine_select(
                out=mask_lo[:], in_=mask_lo[:], compare_op=OP.is_ge, fill=0.0,
                base=0, pattern=[[-1, 128]], channel_multiplier=1,
            )
            # mask_hi: keep p <= f (causal diagonal, jb==8)
            mask_hi = small.tile([128, 128], BF16, tag="mask_hi")
            nc.gpsimd.memset(mask_hi[:], 1.0)
            nc.gpsimd.affine_select(
                out=mask_hi[:], in_=mask_hi[:], compare_op=OP.is_ge, fill=0.0,
                base=0, pattern=[[1, 128]], channel_multiplier=-1,
            )

            # persistent intermediates
            KT_sb = [persist.tile([64, KR], BF16, tag=f"KT{g}", name=f"KT{g}")
                     for g in range(NKV)]
            QT_sb = [persist.tile([64, QR], BF16, tag=f"QT{h}", name=f"QT{h}")
                     for h in range(NH)]
            Vv_sb = [persist.tile([128, NKV, HD + 1], BF16, tag=f"Vv{rt}", name=f"Vv{rt}")
                     for rt in range(NKT)]
            Y_sb = [persist.tile([128, C], BF16, tag=f"Y{it}", name=f"Y{it}")
                    for it in range(NQT)]
            YT_sb = [persist.tile([128, QR], BF16, tag=f"YT{ct}", name=f"YT{ct}")
                     for ct in range(NCT)]

            for rt in range(NKT):
                nc.sync.dma_start(
                    Vv_sb[rt][:, :, HD:HD + 1],
                    valid_d.ap()[rt].unsqueeze(2),
                )

            pools = {}
            # ---- phase A1: K/V/gate, rope+rms K, KT transposes, V gating ----
            with (
                tc.tile_pool(name="pkv", bufs=2, space="PSUM") as pkv,
                tc.tile_pool(name="ptr", bufs=2, space="PSUM") as ptr,
                tc.tile_pool(name="a1sb", bufs=2) as a1sb,
                tc.tile_pool(name="a1sm", bufs=3) as a1sm,
            ):
                pools = {"rr": a1sb, "rtmp": a1sb, "ms": a1sm}
                for rt in range(NKT):
                    rs = slice(rt * 128, (rt + 1) * 128)
                    kp = pkv.tile([128, NKV * HD], F32, tag="kp")
                    vp = pkv.tile([128, NKV * HD], F32, tag="vp")
                    gp = pkv.tile([128, NKV], F32, tag="gp")
                    for ct in range(NCT):
                        st = (ct == 0)
                        sp = (ct == NCT - 1)
                        lhs = xT_sb[ct][:, rs]
                        nc.tensor.matmul(kp[:], lhs, wk_sb[ct][:], start=st, stop=sp)
                        nc.tensor.matmul(vp[:], lhs, wv_sb[ct][:], start=st, stop=sp)
                    nc.tensor.matmul(gp[:], xT_sb[0][0:VEC, rs], wg_sb[:],
                                     start=True, stop=True)
                    # gate = 1 + tanh(z/2)  (== 2*sigmoid(z))
                    gs = a1sm.tile([128, NKV], F32, tag="gs")
                    nc.scalar.activation(gs[:], gp[:], AF.Tanh, scale=0.5)
                    nc.vector.tensor_scalar(gs[:], gs[:], 1.0, None, op0=OP.add)
                    # V = vp + gate * ve   -> Vv[:, :, 0:64] (bf16)
                    tv = a1sb.tile([128, NKV * HD], BF16, tag="tv")
                    tv3 = tv[:].rearrange("p (h d) -> p h d", h=NKV)
                    gb = gs[:].unsqueeze(2).broadcast_to([128, NKV, HD])
                    ve3 = ve_sb[rt][:].rearrange("p (h d) -> p h d", h=NKV)
                    nc.vector.tensor_tensor(tv3, ve3, gb, op=OP.mult)
                    vp3 = vp[:].rearrange("p (h d) -> p h d", h=NKV)
                    nc.vector.tensor_tensor(
                        Vv_sb[rt][:, :, 0:HD], vp3, tv3, op=OP.add
                    )
                    # K: rope + rmsnorm -> kn bf16, then transpose per head
                    kn = a1sb.tile([128, NKV * HD], BF16, tag="kn")
                    _rope_rms(nc, pools, kp[:], cos_sb[rt], sin_sb[rt], kn[:], NKV)
                    for g in range(NKV):
                        tp = ptr.tile([64, 128], BF16, tag="tp")
                        nc.tensor.transpose(
                            tp[:], kn[:, g * HD:(g + 1) * HD], ident[:]
                        )
                        nc.any.tensor_copy(KT_sb[g][:, rs], tp[:])

            # ---- phase A2: Q proj, rope+rms, QT transposes ------------------
            with (
                tc.tile_pool(name="pq", bufs=2, space="PSUM") as pq,
                tc.tile_pool(name="ptr2", bufs=4, space="PSUM") as ptr2,
                tc.tile_pool(name="a2sb", bufs=2) as a2sb,
                tc.tile_pool(name="a2sm", bufs=3) as a2sm,
            ):
                pools = {"rr": a2sb, "rtmp": a2sb, "ms": a2sm}
                for it in range(NQT):
                    rt = (WIN // 128) + it
                    rs = slice(rt * 128, (rt + 1) * 128)
                    qn = a2sb.tile([128, NH * HD], BF16, tag="qn")
                    for half in range(2):
                        qp = pq.tile([128, 512], F32, tag="qp")
                        for ct in range(NCT):
                            nc.tensor.matmul(
                                qp[:], xT_sb[ct][:, rs],
                                wq_sb[ct][:, half * 512:(half + 1) * 512],
                                start=(ct == 0), stop=(ct == NCT - 1),
                            )
                        _rope_rms(nc, pools, qp[:], cos_sb[rt], sin_sb[rt],
                                  qn[:, half * 512:(half + 1) * 512], NH // 2)
                    for h in range(NH):
                        tp = ptr2.tile([64, 128], BF16, tag="tp2")
                        nc.tensor.transpose(
                            tp[:], qn[:, h * HD:(h + 1) * HD], ident[:]
                        )
                        nc.any.tensor_copy(
                            QT_sb[h][:, it * 128:(it + 1) * 128], tp[:]
                        )

            # ---- phase B: attention ----------------------------------------
            with (
                tc.tile_pool(name="pst", bufs=2, space="PSUM") as pst,
                tc.tile_pool(name="pav", bufs=2, space="PSUM") as pav,
                tc.tile_pool(name="bpt", bufs=3) as bpt,
                tc.tile_pool(name="brc", bufs=4) as brc,
            ):
                for h in range(NH):
                    g = h // (NH // NKV)
                    for it in range(NQT):
                        qs = QT_sb[h][:, it * 128:(it + 1) * 128]
                        stp = pst.tile([128, NJB * 128], F32, tag="st")
                        for jb in range(NJB):
                            jt = it + jb
                            nc.tensor.matmul(
                                stp[:, jb * 128:(jb + 1) * 128],
                                KT_sb[g][:, jt * 128:(jt + 1) * 128],
                                qs, start=True, stop=True,
                            )
                        pt = bpt.tile([128, NJB * 128], BF16, tag="pt")
                        nc.scalar.activation(pt[:], stp[:], AF.Exp,
                                             scale=1.0 / np.sqrt(HD))
                        nc.vector.tensor_tensor(
                            pt[:, 0:128], pt[:, 0:128], mask_lo[:], op=OP.mult
                        )
                        nc.vector.tensor_tensor(
                            pt[:, WIN:WIN + 128], pt[:, WIN:WIN + 128],
                            mask_hi[:], op=OP.mult
                        )
                        ov = pav.tile([128, HD + 1], F32, tag="ov")
                        for jb in range(NJB):
                            jt = it + jb
                            nc.tensor.matmul(
                                ov[:], pt[:, jb * 128:(jb + 1) * 128],
                                Vv_sb[jt][:, g, :],
                                start=(jb == 0), stop=(jb == NJB - 1),
                            )
                        rc = brc.tile([128, 1], F32, tag="rc")
                        nc.vector.reciprocal(rc[:], ov[:, HD:HD + 1])
                        nc.vector.tensor_scalar(
                            Y_sb[it][:, h * HD:(h + 1) * HD], ov[:, 0:HD],
                            rc[:], None, op0=OP.mult,
                        )

            # ---- phase C: YT transposes + output projection -----------------
            with (
                tc.tile_pool(name="pyt", bufs=4, space="PSUM") as pyt,
                tc.tile_pool(name="ppr", bufs=4, space="PSUM") as ppr,
                tc.tile_pool(name="cout", bufs=2) as cout,
            ):
                for it in range(NQT):
                    for ct in range(NCT):
                        tp = pyt.tile([128, 128], BF16, tag="typ")
                        nc.tensor.transpose(
                            tp[:], Y_sb[it][:, ct * 128:(ct + 1) * 128], ident[:]
                        )
                        nc.any.tensor_copy(
                            YT_sb[ct][:, it * 128:(it + 1) * 128], tp[:]
                        )
                    ob = cout.tile([128, C], F32, tag="ob")
                    for half in range(2):
                        pr = ppr.tile([128, 512], F32, tag="pr")
                        for ct in range(NCT):
                            nc.tensor.matmul(
                                pr[:],
                                YT_sb[ct][:, it * 128:(it + 1) * 128],
                                wp_sb[ct][:, half * 512:(half + 1) * 512],
                                start=(ct == 0), stop=(ct == NCT - 1),
                            )
                        nc.any.tensor_copy(
                            ob[:, half * 512:(half + 1) * 512], pr[:]
                        )
                    nc.sync.dma_start(
                        y_d.ap()[it * 128:(it + 1) * 128, :], ob[:]
                    )
    nc.compile()
    return nc


_CACHED = {}


def _get_program():
    if "nc" not in _CACHED:
        _CACHED["nc"] = build_program()
    return _CACHED["nc"]


def _prep_inputs(x, ve, cos, sin, Wq, Wk, Wv, Wproj, Wgate):
    bf = ml_dtypes.bfloat16
    wq = np.ascontiguousarray(Wq.astype(bf))
    wk = np.ascontiguousarray(Wk.astype(bf))
    wv = np.ascontiguousarray(Wv.astype(bf))
    wp = np.ascontiguousarray(Wproj.astype(bf))
    wg = np.ascontiguousarray(Wgate.astype(bf))
    cos2 = cos[0, :, 0, :]
    sin2 = sin[0, :, 0, :]
    in_maps = []
    for c in range(N_CORES):
        b, j = divmod(c, N_CORES // B)
        q0 = QR * j
        k0 = q0 - WIN
        pad = max(0, -k0)
        lo = max(0, k0)
        xTc = np.zeros((C, KR), dtype=bf)
        xTc[:, pad:] = x[b, lo:q0 + QR, :].T.astype(bf)
        vec = np.zeros((KR, NKV * HD), dtype=bf)
        vec[pad:] = ve[b, lo:q0 + QR, :].astype(bf)
        cosc = np.zeros((KR, 32), dtype=np.float32)
        cosc[pad:] = cos2[lo:q0 + QR]
        sinc = np.zeros((KR, 32), dtype=np.float32)
        sinc[pad:] = sin2[lo:q0 + QR]
        validc = np.zeros((KR,), dtype=bf)
        validc[pad:] = 1.0
        validc = np.ascontiguousarray(
            np.broadcast_to(validc.reshape(NKT, 128, 1), (NKT, 128, NKV))
        )
        in_maps.append({
            "xT": np.ascontiguousarray(xTc),
            "ve": np.ascontiguousarray(vec),
            "cos": cosc, "sin": sinc,
            "wq": wq, "wk": wk, "wv": wv, "wproj": wp, "wgate": wg,
            "valid": validc,
        })
    return in_maps


def kernel(x, ve, cos, sin, Wq, Wk, Wv, Wproj, Wgate, window_size, **_):
    assert int(window_size) == WIN, f"kernel hardcodes window={WIN}"
    x = np.asarray(x, dtype=np.float32)
    ve = np.asarray(ve, dtype=np.float32)
    cos = np.asarray(cos, dtype=np.float32)
    sin = np.asarray(sin, dtype=np.float32)
    in_maps = _prep_inputs(x, ve, cos, sin,
                           np.asarray(Wq, np.float32), np.asarray(Wk, np.float32),
                           np.asarray(Wv, np.float32), np.asarray(Wproj, np.float32),
                           np.asarray(Wgate, np.float32))
    nc = _get_program()
    res = run_bass_kernel_spmd(nc, in_maps, list(range(N_CORES)))
    out = np.empty((B, T, C), dtype=np.float32)
    for c in range(N_CORES):
        b, j = divmod(c, N_CORES // B)
        out[b, QR * j:QR * (j + 1), :] = res.results[c]["y"]
    return out


if __name__ == "__main__":
    rng = np.random.default_rng(0)
    ins = {
        "x": rng.standard_normal((B, T, C), dtype=np.float32),
        "ve": rng.standard_normal((B, T, NKV * HD), dtype=np.float32),
        "cos": rng.standard_normal((1, T, 1, 32), dtype=np.float32),
        "sin": rng.standard_normal((1, T, 1, 32), dtype=np.float32),
        "Wq": rng.standard_normal((C, NH * HD), dtype=np.float32) * 0.02,
        "Wk": rng.standard_normal((C, NKV * HD), dtype=np.float32) * 0.02,
        "Wv": rng.standard_normal((C, NKV * HD), dtype=np.float32) * 0.02,
        "Wproj": rng.standard_normal((C, C), dtype=np.float32) * 0.02,
        "Wgate": rng.standard_normal((VEC, NKV), dtype=np.float32) * 0.02,
        "window_size": 1024,
    }
    y = kernel(**ins)
    print("ran, out shape", y.shape, "mean", float(np.abs(y).mean()))
